# revision 7
# baseline (speedup 1.0000x reference)
"""Trainium2 Bass kernel for nn_Decoder (single-step attention decoder).

Sharding: data-parallel over batch B=64 (8 rows/core) for the dominant
attention compute; AllGather of context; tensor-parallel (gate-sliced) LSTM;
vocab-parallel output projection with host-side concat.

Self-contained: hardcodes all shapes; builds + jit-compiles once per process.
"""
import numpy as np
import ml_dtypes
from contextlib import ExitStack

import jax
import concourse.bass as bass
import concourse.tile as tile
import concourse.mybir as mybir
from concourse import bass2jax

BF = ml_dtypes.bfloat16
f32 = mybir.dt.float32
bf16 = mybir.dt.bfloat16
f32r = mybir.dt.float32r
AF = mybir.ActivationFunctionType
AX = mybir.AxisListType

N_CORES = 8
B, S, V, D, H, E, A = 64, 1024, 32000, 1024, 1024, 1024, 1024
BPC = B // N_CORES          # batch rows per core
VPC = V // N_CORES          # vocab cols per core
HPC = H // N_CORES          # hidden slice per core
GS = 4 * HPC                # gate-slice width (512)
NT = 500                    # logits N-tile (4000 = 8 x 500)


# ---------------------------------------------------------------------------
# walrus workaround: this toolchain's ISA structs accept only ONE sync-wait
# per instruction; Tile attaches several. Move excess waits onto injected
# same-engine NoOps placed right before the over-subscribed instruction.
def _split_sync_waits(nc, max_waits=1):
    for f in nc.m.functions:
        for bb in f.blocks:
            new_list = []
            for inst in bb.instructions:
                si = inst.sync_info
                if si is not None and len(si.on_wait) > max_waits:
                    waits = list(si.on_wait)
                    extra, keep = waits[:-max_waits], waits[-max_waits:]
                    for i in range(0, len(extra), max_waits):
                        new_list.append(mybir.InstNoOp(
                            name=f"{inst.name}-wsplit{i}",
                            engine=inst.engine,
                            bass_nofuse=True,
                            sync_info=mybir.SyncInfo(
                                on_wait=extra[i:i + max_waits], on_update=[]),
                        ))
                    inst.sync_info = mybir.SyncInfo(
                        on_wait=keep, on_update=list(si.on_update))
                new_list.append(inst)
            bb.instructions = new_list


# ---------------------------------------------------------------------------
def _build():
    nc = bass.Bass("TRN2", target_bir_lowering=False, debug=False,
                   num_devices=N_CORES)

    def inp(name, shape, dt):
        return nc.dram_tensor(name, shape, dt, kind="ExternalInput")

    enct = inp("enct", [BPC, E, S], bf16)          # encoder [b][e][s]
    encn = inp("encn", [BPC, S, E], bf16)          # encoder [b][s][e]
    waet = inp("waet", [E, A], bf16)               # Wa_e.T
    waht = inp("waht", [H, A], bf16)               # Wa_h.T
    h0t1m = inp("h0t1m", [H, BPC], bf16)           # h0[1].T my-batch cols
    qbias = inp("qbias", [A], f32)                 # ba_h + ba_e
    va_d = inp("va_d", [A], bf16)
    embt = inp("embt", [D, B], bf16)               # embedded.T (full batch)
    h0t0 = inp("h0t0", [H, B], bf16)
    h0t1 = inp("h0t1", [H, B], bf16)
    wih0t = inp("wih0t", [D + E, GS], bf16)        # Wih0 slice^T
    whh0t = inp("whh0t", [H, GS], bf16)
    wih1t = inp("wih1t", [H, GS], bf16)
    whh1t = inp("whh1t", [H, GS], bf16)
    b0_d = inp("b0_d", [1, GS], f32r)              # bih0+bhh0 slice
    b1_d = inp("b1_d", [1, GS], f32r)
    c00 = inp("c00", [B, HPC], f32)                # c0[0] slice
    c01 = inp("c01", [B, HPC], f32)
    woutt = inp("woutt", [H, VPC], bf16)           # Wout slice^T
    bout_d = inp("bout_d", [1, VPC], f32r)
    onesM = inp("onesM", [1, B], f32r)
    ident = inp("ident", [128, 128], bf16)

    def outp(name, shape):
        return nc.dram_tensor(name, shape, f32, kind="ExternalOutput")

    o_logits = outp("o_logits", [B, VPC])
    o_attnt = outp("o_attnt", [128, BPC, 8])       # attn^T [p][b][chunk]
    o_h1 = outp("o_h1", [B, HPC])
    o_h2 = outp("o_h2", [B, HPC])
    o_c1 = outp("o_c1", [B, HPC])
    o_c2 = outp("o_c2", [B, HPC])

    with tile.TileContext(nc) as tc:
        with ExitStack() as ctx:
            # ---- static pools (whole kernel) ----
            const = ctx.enter_context(tc.tile_pool(name="const", bufs=1))
            dram = ctx.enter_context(tc.tile_pool(name="dram", bufs=1, space="DRAM"))

            waet_sb = const.tile([128, 8, A], bf16)
            nc.sync.dma_start(waet_sb[:], waet.ap().rearrange("(c p) a -> p c a", p=128))
            va_sb = const.tile([128, 8], bf16)
            nc.sync.dma_start(va_sb[:], va_d.ap().rearrange("(c p) -> p c", p=128))
            qbias_sb = const.tile([128, 8], f32)
            nc.sync.dma_start(qbias_sb[:], qbias.ap().rearrange("(c p) -> p c", p=128))
            ident_sb = const.tile([128, 128], bf16)
            nc.sync.dma_start(ident_sb[:], ident.ap())
            onesM_sb = const.tile([1, B], f32r)
            nc.sync.dma_start(onesM_sb[:], onesM.ap())

            # ---- q = h0[-1] @ Wa_h.T + (ba_h + ba_e), per-core batch ----
            qtot_sb = const.tile([128, 8 * BPC], f32)   # [p][at*8+b]
            with ExitStack() as qctx:
                qpool = qctx.enter_context(tc.tile_pool(name="qpool", bufs=1))
                qps_pool = qctx.enter_context(
                    tc.tile_pool(name="qps", bufs=1, space="PSUM"))
                waht_sb = qpool.tile([128, 8, A], bf16)
                nc.sync.dma_start(waht_sb[:], waht.ap().rearrange("(c p) a -> p c a", p=128))
                h0t1m_sb = qpool.tile([128, 8, BPC], bf16)
                nc.sync.dma_start(h0t1m_sb[:], h0t1m.ap().rearrange("(c p) b -> p c b", p=128))
                q_ps = qps_pool.tile([128, 8 * BPC], f32)
                for at in range(8):
                    for hc in range(8):
                        nc.tensor.matmul(
                            q_ps[:, at * BPC:(at + 1) * BPC],
                            waht_sb[:, hc, at * 128:(at + 1) * 128],
                            h0t1m_sb[:, hc, :],
                            start=(hc == 0), stop=(hc == 7))
                for at in range(8):
                    nc.scalar.activation(
                        qtot_sb[:, at * BPC:(at + 1) * BPC],
                        q_ps[:, at * BPC:(at + 1) * BPC],
                        AF.Identity, bias=qbias_sb[:, at:at + 1])

            # ---- phase A: attention, per local batch row ----
            attnt_all = const.tile([128, BPC, 8], f32)
            ctx_sb = const.tile([1, BPC * E], bf16)

            actx = ExitStack()
            apool = actx.enter_context(tc.tile_pool(name="apool", bufs=2))
            enct_pool = actx.enter_context(tc.tile_pool(name="enctp", bufs=2))
            encn_pool = actx.enter_context(tc.tile_pool(name="encnp", bufs=2))
            t2_pool = actx.enter_context(tc.tile_pool(name="t2p", bufs=3))
            row_pool = actx.enter_context(tc.tile_pool(name="rowp", bufs=2))
            kps_pool = actx.enter_context(tc.tile_pool(name="kps", bufs=2, space="PSUM"))
            sps_pool = actx.enter_context(tc.tile_pool(name="sps", bufs=1, space="PSUM"))
            cps_pool = actx.enter_context(tc.tile_pool(name="cps", bufs=1, space="PSUM"))

            for b in range(BPC):
                enct_sb = enct_pool.tile([128, 8, S], bf16, name="enct_sb")
                nc.sync.dma_start(
                    enct_sb[:], enct.ap()[b].rearrange("(c p) s -> p c s", p=128))
                encn_sb = encn_pool.tile([128, 8, E], bf16, name="encn_sb")
                nc.sync.dma_start(
                    encn_sb[:], encn.ap()[b].rearrange("(c p) e -> p c e", p=128))

                score_ps = sps_pool.tile([1, S], f32, name="score_ps")
                for sh in range(2):
                    for at in range(8):
                        kps = kps_pool.tile([128, 512], f32, name="kps")
                        for ec in range(8):
                            nc.tensor.matmul(
                                kps[:],
                                waet_sb[:, ec, at * 128:(at + 1) * 128],
                                enct_sb[:, ec, sh * 512:(sh + 1) * 512],
                                start=(ec == 0), stop=(ec == 7))
                        t2 = t2_pool.tile([128, 512], bf16, name="t2")
                        nc.scalar.activation(
                            t2[:], kps[:], AF.Tanh,
                            bias=qtot_sb[:, at * BPC + b:at * BPC + b + 1])
                        nc.tensor.matmul(
                            score_ps[0:1, sh * 512:(sh + 1) * 512],
                            va_sb[:, at:at + 1], t2[:],
                            start=(at == 0), stop=(at == 7))

                # softmax (no max-subtraction needed: |score| <= sum|va| ~ 16)
                esc = row_pool.tile([1, S], f32, name="esc")
                nc.scalar.activation(esc[:], score_ps[:], AF.Exp)
                zsum = row_pool.tile([1, 1], f32, name="zsum")
                nc.vector.reduce_sum(zsum[:], esc[:], axis=AX.X)
                zrec = row_pool.tile([1, 1], f32, name="zrec")
                nc.vector.reciprocal(zrec[:], zsum[:])
                attn_row = row_pool.tile([1, S], f32, name="attn_row")
                nc.vector.tensor_scalar_mul(attn_row[:], esc[:], zrec[:])
                # scatter row -> columns via DRAM bounce
                attn_d = dram.tile([1, S], f32, name="attn_d", bufs=2)
                nc.sync.dma_start(attn_d[:], attn_row[:])
                nc.sync.dma_start(
                    attnt_all[:, b, :],
                    attn_d[0, :].rearrange("(c p) -> p c", p=128))
                ctxw = apool.tile([128, 8], bf16, name="ctxw")
                nc.vector.tensor_copy(ctxw[:], attnt_all[:, b, :])

                # context = attn @ enc   [1, E]
                ctx_ps = cps_pool.tile([1, E], f32, name="ctx_ps")
                for eh in range(2):
                    for sc in range(8):
                        nc.tensor.matmul(
                            ctx_ps[0:1, eh * 512:(eh + 1) * 512],
                            ctxw[:, sc:sc + 1],
                            encn_sb[:, sc, eh * 512:(eh + 1) * 512],
                            start=(sc == 0), stop=(sc == 7))
                nc.vector.tensor_copy(ctx_sb[0:1, b * E:(b + 1) * E], ctx_ps[:])

            nc.sync.dma_start(o_attnt.ap(), attnt_all[:])
            actx.close()

            # ---- AllGather context ----
            ctx_in = dram.tile([BPC, E], bf16, name="ctx_in")
            ctx_out = dram.tile([B, E], bf16, name="ctx_out", addr_space="Shared")
            nc.gpsimd.dma_start(ctx_in[:], ctx_sb[0, :].rearrange('(b e) -> b e', b=BPC))
            nc.gpsimd.collective_compute(
                "AllGather", mybir.AluOpType.bypass,
                replica_groups=[list(range(N_CORES))],
                ins=[ctx_in.opt()], outs=[ctx_out.opt()])

            # ---- LSTM weights (prefetched during phase A by DMA order) ----
            lstm = ctx.enter_context(tc.tile_pool(name="lstm", bufs=1))
            wih0t_sb = lstm.tile([128, 16, GS], bf16)
            nc.sync.dma_start(wih0t_sb[:], wih0t.ap().rearrange("(c p) n -> p c n", p=128))
            whh0t_sb = lstm.tile([128, 8, GS], bf16)
            nc.sync.dma_start(whh0t_sb[:], whh0t.ap().rearrange("(c p) n -> p c n", p=128))
            wih1t_sb = lstm.tile([128, 8, GS], bf16)
            nc.sync.dma_start(wih1t_sb[:], wih1t.ap().rearrange("(c p) n -> p c n", p=128))
            whh1t_sb = lstm.tile([128, 8, GS], bf16)
            nc.sync.dma_start(whh1t_sb[:], whh1t.ap().rearrange("(c p) n -> p c n", p=128))
            embt_sb = lstm.tile([128, 8, B], bf16)
            nc.sync.dma_start(embt_sb[:], embt.ap().rearrange("(c p) b -> p c b", p=128))
            h0t0_sb = lstm.tile([128, 8, B], bf16)
            nc.sync.dma_start(h0t0_sb[:], h0t0.ap().rearrange("(c p) b -> p c b", p=128))
            h0t1_sb = lstm.tile([128, 8, B], bf16)
            nc.sync.dma_start(h0t1_sb[:], h0t1.ap().rearrange("(c p) b -> p c b", p=128))
            b0_sb = lstm.tile([1, GS], f32r)
            nc.sync.dma_start(b0_sb[:], b0_d.ap())
            b1_sb = lstm.tile([1, GS], f32r)
            nc.sync.dma_start(b1_sb[:], b1_d.ap())
            c00_sb = lstm.tile([B, HPC], f32)
            nc.sync.dma_start(c00_sb[:], c00.ap())
            c01_sb = lstm.tile([B, HPC], f32)
            nc.sync.dma_start(c01_sb[:], c01.ap())
            bout_sb = lstm.tile([1, VPC], f32r)
            nc.sync.dma_start(bout_sb[:], bout_d.ap())

            bpool = ctx.enter_context(tc.tile_pool(name="bpool", bufs=2))
            bps = ctx.enter_context(tc.tile_pool(name="bps", bufs=2, space="PSUM"))

            # ctx_full -> transposed bf16 chunks
            ctx_full = bpool.tile([B, E], bf16, name="ctx_full")
            nc.sync.dma_start(ctx_full[:], ctx_out[:])
            xT_ct = lstm.tile([128, 8, B], bf16)
            for hc in range(8):
                tp = bps.tile([128, B], bf16, name="tp", tag="tps")
                nc.tensor.transpose(tp[:], ctx_full[:, hc * 128:(hc + 1) * 128], ident_sb[0:B, 0:B])
                nc.vector.tensor_copy(xT_ct[:, hc, :], tp[:])

            def lstm_layer(x_chunks, wih_sb, n_in_ch, h0t_sb, whh_sb, bias_sb,
                           c0_sb, o_h, o_c, h1t_in, h1t_out, hT_sb, lname):
                gps = bps.tile([B, GS], f32, name=f"gps{lname}", tag="gps")
                k = 0
                n_tot = n_in_ch + 8 + 1
                for i in range(n_in_ch):
                    nc.tensor.matmul(gps[:], x_chunks(i), wih_sb[:, i, :],
                                     start=(k == 0), stop=False)
                    k += 1
                for hc in range(8):
                    nc.tensor.matmul(gps[:], h0t_sb[:, hc, :], whh_sb[:, hc, :],
                                     start=False, stop=False)
                    k += 1
                nc.tensor.matmul(gps[:], onesM_sb[:], bias_sb[:],
                                 start=False, stop=True)
                sig_i = bpool.tile([B, HPC], f32, name=f"sgi{lname}")
                nc.scalar.activation(sig_i[:], gps[:, 0:HPC], AF.Sigmoid)
                sig_f = bpool.tile([B, HPC], f32, name=f"sgf{lname}")
                nc.scalar.activation(sig_f[:], gps[:, HPC:2 * HPC], AF.Sigmoid)
                tan_g = bpool.tile([B, HPC], f32, name=f"tng{lname}")
                nc.scalar.activation(tan_g[:], gps[:, 2 * HPC:3 * HPC], AF.Tanh)
                sig_o = bpool.tile([B, HPC], f32, name=f"sgo{lname}")
                nc.scalar.activation(sig_o[:], gps[:, 3 * HPC:4 * HPC], AF.Sigmoid)
                t_fc = bpool.tile([B, HPC], f32, name=f"tfc{lname}")
                nc.vector.tensor_mul(t_fc[:], sig_f[:], c0_sb[:])
                t_ig = bpool.tile([B, HPC], f32, name=f"tig{lname}")
                nc.vector.tensor_mul(t_ig[:], sig_i[:], tan_g[:])
                c1 = bpool.tile([B, HPC], f32, name=f"c1{lname}")
                nc.vector.tensor_add(c1[:], t_fc[:], t_ig[:])
                nc.sync.dma_start(o_c.ap(), c1[:])
                tc1 = bpool.tile([B, HPC], f32, name=f"tc1{lname}")
                nc.scalar.activation(tc1[:], c1[:], AF.Tanh)
                h1 = bpool.tile([B, HPC], f32, name=f"h1{lname}")
                nc.vector.tensor_mul(h1[:], sig_o[:], tc1[:])
                nc.sync.dma_start(o_h.ap(), h1[:])
                h1b = bpool.tile([B, HPC], bf16, name=f"h1b{lname}")
                nc.vector.tensor_copy(h1b[:], h1[:])
                htp = bps.tile([128, B], bf16, name=f"htp{lname}", tag="tps")
                nc.tensor.transpose(htp[:], h1b[:], ident_sb[0:B, 0:B])
                ht_sb = bpool.tile([128, B], bf16, name=f"ht{lname}")
                nc.vector.tensor_copy(ht_sb[:], htp[:])
                nc.gpsimd.dma_start(h1t_in[:], ht_sb[:])
                nc.gpsimd.collective_compute(
                    "AllGather", mybir.AluOpType.bypass,
                    replica_groups=[list(range(N_CORES))],
                    ins=[h1t_in.opt()], outs=[h1t_out.opt()])
                nc.sync.dma_start(
                    hT_sb[:], h1t_out.rearrange("(c p) b -> p c b", p=128))

            h1t_in = dram.tile([128, B], bf16, name="h1t_in")
            h1t_out = dram.tile([H, B], bf16, name="h1t_out", addr_space="Shared")
            h1T_sb = lstm.tile([128, 8, B], bf16)
            lstm_layer(lambda i: embt_sb[:, i, :] if i < 8 else xT_ct[:, i - 8, :],
                       wih0t_sb, 16, h0t0_sb, whh0t_sb, b0_sb, c00_sb,
                       o_h1, o_c1, h1t_in, h1t_out, h1T_sb, "L0")

            h2t_in = dram.tile([128, B], bf16, name="h2t_in")
            h2t_out = dram.tile([H, B], bf16, name="h2t_out", addr_space="Shared")
            h2T_sb = lstm.tile([128, 8, B], bf16)
            lstm_layer(lambda i: h1T_sb[:, i, :],
                       wih1t_sb, 8, h0t1_sb, whh1t_sb, b1_sb, c01_sb,
                       o_h2, o_c2, h2t_in, h2t_out, h2T_sb, "L1")

            # ---- logits: vocab-sharded, stream Wout slice in 4 groups ----
            wo_pool = ctx.enter_context(tc.tile_pool(name="wop", bufs=2))
            lg_pool = ctx.enter_context(tc.tile_pool(name="lgp", bufs=3))
            lg_ps_pool = ctx.enter_context(tc.tile_pool(name="lgps", bufs=3, space="PSUM"))
            for g in range(4):
                wo_t = wo_pool.tile([128, 8, 1000], bf16, name="wo_t")
                nc.sync.dma_start(
                    wo_t[:],
                    woutt.ap()[:, g * 1000:(g + 1) * 1000]
                    .rearrange("(c p) n -> p c n", p=128))
                for half in range(2):
                    off = g * 1000 + half * NT
                    lg_ps = lg_ps_pool.tile([B, NT], f32, name="lg_ps")
                    for kc in range(8):
                        nc.tensor.matmul(
                            lg_ps[:], h2T_sb[:, kc, :],
                            wo_t[:, kc, half * NT:(half + 1) * NT],
                            start=(kc == 0), stop=False)
                    nc.tensor.matmul(lg_ps[:], onesM_sb[:],
                                     bout_sb[0:1, off:off + NT],
                                     start=False, stop=True)
                    lg_sb = lg_pool.tile([B, NT], f32, name="lg_sb")
                    nc.vector.tensor_copy(lg_sb[:], lg_ps[:])
                    nc.sync.dma_start(o_logits.ap()[:, off:off + NT], lg_sb[:])

    _split_sync_waits(nc)
    return nc


# ---------------------------------------------------------------------------
# host-side: input prep, cached jit runner, output assembly
_CACHE = {}


def _prepare_in_maps(inputs):
    g = {k: np.asarray(v) for k, v in inputs.items()}
    enc = g["encoder_outputs"].astype(np.float32)
    emb = g["emb"]
    idx = g["input_step"].astype(np.int64)[:, 0]
    embedded = np.asarray(emb)[idx]                      # [B, D]
    embt = np.ascontiguousarray(embedded.T).astype(BF)
    h0, c0 = g["h0"], g["c0"]
    h0t0 = np.ascontiguousarray(h0[0].T).astype(BF)
    h0t1 = np.ascontiguousarray(h0[1].T).astype(BF)
    waet = np.ascontiguousarray(g["Wa_e"].T).astype(BF)
    waht = np.ascontiguousarray(g["Wa_h"].T).astype(BF)
    qbias = (g["ba_h"] + g["ba_e"]).astype(np.float32)
    va = g["va"].astype(BF)
    ident = np.eye(128, dtype=np.float32).astype(BF)
    onesM = np.ones((1, B), dtype=np.float32)
    b0 = (g["bih0"] + g["bhh0"]).astype(np.float32)
    b1 = (g["bih1"] + g["bhh1"]).astype(np.float32)

    wih0 = g["Wih0"]; whh0 = g["Whh0"]; wih1 = g["Wih1"]; whh1 = g["Whh1"]
    wout = g["Wout"]; bout = g["bout"].astype(np.float32)

    in_maps = []
    for c in range(N_CORES):
        bs = slice(c * BPC, (c + 1) * BPC)
        hs = np.concatenate([np.arange(gq * H + c * HPC, gq * H + (c + 1) * HPC)
                             for gq in range(4)])
        enc_c = enc[bs]
        enct_c = np.ascontiguousarray(enc_c.transpose(0, 2, 1)).astype(BF)
        encn_c = enc_c.astype(BF)
        m = dict(
            enct=enct_c, encn=encn_c, waet=waet, waht=waht,
            h0t1m=np.ascontiguousarray(h0t1[:, bs]),
            qbias=qbias, va_d=va, embt=embt, h0t0=h0t0, h0t1=h0t1,
            wih0t=np.ascontiguousarray(wih0[hs, :].T).astype(BF),
            whh0t=np.ascontiguousarray(whh0[hs, :].T).astype(BF),
            wih1t=np.ascontiguousarray(wih1[hs, :].T).astype(BF),
            whh1t=np.ascontiguousarray(whh1[hs, :].T).astype(BF),
            b0_d=b0[hs][None, :], b1_d=b1[hs][None, :],
            c00=np.ascontiguousarray(c0[0][:, c * HPC:(c + 1) * HPC]).astype(np.float32),
            c01=np.ascontiguousarray(c0[1][:, c * HPC:(c + 1) * HPC]).astype(np.float32),
            woutt=np.ascontiguousarray(wout[c * VPC:(c + 1) * VPC, :].T).astype(BF),
            bout_d=bout[c * VPC:(c + 1) * VPC][None, :],
            onesM=onesM, ident=ident,
        )
        in_maps.append(m)
    return in_maps


def _get_runner():
    """Build nc + a reusable sharded jit callable (compile once)."""
    if "runner" in _CACHE:
        return _CACHE["runner"]
    from jax.sharding import Mesh, PartitionSpec
    from jax.experimental.shard_map import shard_map

    nc = _build()
    bass2jax.install_neuronx_cc_hook()

    partition_name = nc.partition_id_tensor.name if nc.partition_id_tensor else None
    in_names, out_names, out_avals, zero_outs = [], [], [], []
    for alloc in nc.m.functions[0].allocations:
        if not isinstance(alloc, mybir.MemoryLocationSet):
            continue
        name = alloc.memorylocations[0].name
        if alloc.kind == "ExternalInput":
            if name != partition_name:
                in_names.append(name)
        elif alloc.kind == "ExternalOutput":
            out_names.append(name)
            shape = tuple(alloc.tensor_shape)
            dtype = mybir.dt.np(alloc.dtype)
            out_avals.append(jax.core.ShapedArray(shape, dtype))
            zero_outs.append(np.zeros(shape, dtype))
    n_params = len(in_names)
    n_outs = len(out_avals)
    all_in_names = list(in_names) + list(out_names)
    if partition_name is not None:
        all_in_names.append(partition_name)

    def _body(*args):
        operands = list(args)
        if partition_name is not None:
            operands.append(bass2jax.partition_id_tensor())
        outs = bass2jax._bass_exec_p.bind(
            *operands,
            out_avals=tuple(out_avals),
            in_names=tuple(all_in_names),
            out_names=tuple(out_names),
            lowering_input_output_aliases=(),
            sim_require_finite=True,
            sim_require_nnan=True,
            nc=nc,
        )
        return tuple(outs)

    devices = jax.devices()[:N_CORES]
    mesh = Mesh(np.asarray(devices), ("core",))
    in_specs = (PartitionSpec("core"),) * (n_params + n_outs)
    out_specs = (PartitionSpec("core"),) * n_outs
    donate = tuple(range(n_params, n_params + n_outs))
    sharded = jax.jit(
        shard_map(_body, mesh=mesh, in_specs=in_specs, out_specs=out_specs,
                  check_rep=False),
        donate_argnums=donate, keep_unused=True)

    def run(in_maps):
        concat_in = [
            np.concatenate([np.asarray(in_maps[c][nm]) for c in range(N_CORES)], axis=0)
            for nm in in_names
        ]
        concat_zeros = [
            np.zeros((N_CORES * z.shape[0], *z.shape[1:]), z.dtype)
            for z in zero_outs
        ]
        out_arrs = sharded(*concat_in, *concat_zeros)
        return [
            {nm: np.asarray(out_arrs[i]).reshape(N_CORES, *out_avals[i].shape)[c]
             for i, nm in enumerate(out_names)}
            for c in range(N_CORES)
        ]

    _CACHE["runner"] = run
    return run


def _assemble(results):
    logits = np.concatenate([r["o_logits"] for r in results], axis=1)
    attn = np.concatenate(
        [r["o_attnt"].transpose(1, 2, 0).reshape(BPC, S) for r in results], axis=0)
    h1 = np.concatenate([r["o_h1"] for r in results], axis=1)
    h2 = np.concatenate([r["o_h2"] for r in results], axis=1)
    c1 = np.concatenate([r["o_c1"] for r in results], axis=1)
    c2 = np.concatenate([r["o_c2"] for r in results], axis=1)
    h_new = np.stack([h1, h2])
    c_new = np.stack([c1, c2])
    return logits, h_new, c_new, attn


def kernel(**inputs):
    in_maps = _prepare_in_maps(inputs)
    run = _get_runner()
    results = run(in_maps)
    return _assemble(results)
